# revision 28
# baseline (speedup 1.0000x reference)
"""HT2IM scatter kernel for Trainium2 (8 NeuronCores, SPMD).

Math: out[ch, p] += ht[ch, q] * w  for each vote (q=ht_index[v], p=im_index[v]),
      ch over B*C=256 channels, q < 10980 (HT pixels), p < 16384 (IM pixels).

Device formulation: out[ch, p] = sum_q ht_T[q, ch] * S[q, p] with the sparse
vote-aggregate matrix S[q, p] = sum_v w_v [q_v=q][p_v=p], sharded over output
pixels (2048 columns per core, full ht_T per core).

To run the PE at the fp8 DoubleRow rate (0.5 cycles/row, 2x bf16) while
keeping rel err ~4e-3, the product is decomposed into three e4m3 DoubleRow
passes per stripe pair (k = 256 per instruction):
  S cells are 8-bit-uniform coded: X = round(128*x), hi = (X>>4)/8,
  lo = (X&15)/8 (both exactly representable in e4m3) so that
  ht*x ~= ht*hi + (ht/16)*lo.  ht is split ht = hhi + hlo (e4m3 + e4m3
  residual).  The three products accumulated in fp32 PSUM:
      P1 = hhi @ S_hi     P2 = hlo @ S_hi     P3 = (hhi/16) @ S_lo
(the dropped hlo-x-S_lo term is ~1e-4 of scale).

S delivery is SPARSE: each 64B scatter row carries 32 int16 cells, a cell
being the (hi, lo) byte pair of one (q, p) entry; tiles are [128, 2048 cells]
per q-stripe in a 16-slot SBUF ring, zeroed by DVE (int16 4x memset) and
filled by GPSIMD SWDGE dma_scatter_add (int16 CCE, rows pre-deduped on host
so every add is 0+x = exact).  One scatter call fills 4 stripes using the
parity-split dual destination.  The PE reads the hi/lo planes of the same
tiles via stride-2 access patterns ([128, 2, 512, 1] rhs, digit-indexed).

Engine budget per core: PE 110us (1032 DoubleRow matmuls), DMA ~105us
(scatter rows 8.4MB + row/idx streams + 3x e4m3 ht + bf16 out), GPSIMD 64us,
DVE 47us - PE-bound with the sparse stream just underneath.
"""

import numpy as np
import ml_dtypes

import concourse.bass as bass
from concourse import bacc
from concourse import mybir
from concourse import bass_utils

BF16 = ml_dtypes.bfloat16
E4M3 = ml_dtypes.float8_e4m3

B, C = 4, 64
CH = B * C                  # 256 channels
HT_H, HT_W = 183, 60
Q = HT_H * HT_W             # 10980
QP = 11008                  # padded to 86*128
NSTRIPE = 86
NPAIR = 43
IM_H, IM_W = 128, 128
P = IM_H * IM_W             # 16384
NCORES = 8
PSL = P // NCORES           # 2048 pixel columns per core
NRING = 16                  # S-tile ring depth (stripes)
NCALL = 22                  # scatter calls (4 stripes each, last has 2)
CAP = 6144                  # scatter row capacity per call
ELEM = 32                   # int16 cells per scatter row (64 B)
# ht chunk boundaries (stripe pairs): small first chunks for pipeline fill
HTB = [0, 1, 2, 6, 10, 14, 18, 22, 26, 30, 34, 38, 43]

_cache = {}


def _build_nc():
    if "nc" in _cache:
        return _cache["nc"]
    f32 = mybir.dt.float32
    bf16 = mybir.dt.bfloat16
    f8e4 = mybir.dt.float8e4
    i16 = mybir.dt.int16
    i32 = mybir.dt.int32

    nc = bacc.Bacc(None, target_bir_lowering=False)
    hta_d = nc.dram_tensor("hta", [128, NSTRIPE, 3, CH], f8e4, kind="ExternalInput")
    wrows_d = nc.dram_tensor("wrows", [NCALL, 128, CAP // 128, ELEM], i16,
                             kind="ExternalInput")
    idxs_d = nc.dram_tensor("idxs", [NCALL, 128, CAP // 16], i16,
                            kind="ExternalInput")
    cnts_d = nc.dram_tensor("cnts", [1, 32], i32, kind="ExternalInput")
    out_d = nc.dram_tensor("out", [2, 128, PSL], bf16, kind="ExternalOutput")

    from contextlib import ExitStack
    ctx = ExitStack()
    with ctx:
        hta_sb = ctx.enter_context(
            nc.sbuf_tensor("k_hta", [128, NSTRIPE, 3, CH], f8e4))
        s_sb = ctx.enter_context(
            nc.sbuf_tensor("k_ssb", [128, NRING, PSL, 2], f8e4))
        s16 = s_sb.bitcast(i16)            # [128, NRING, PSL, 1]
        dummy = ctx.enter_context(nc.sbuf_tensor("k_dum", [128, 2, PSL, 1], i16))
        wb = ctx.enter_context(nc.sbuf_tensor("k_wb", [128, 4, CAP // 128, ELEM], i16))
        ib = ctx.enter_context(nc.sbuf_tensor("k_ib", [128, 4, CAP // 16], i16))
        cnt_sb = ctx.enter_context(nc.sbuf_tensor("k_cnt", [1, 32], i32))
        st0 = ctx.enter_context(nc.sbuf_tensor("k_st0", [128, PSL], bf16))
        st1 = ctx.enter_context(nc.sbuf_tensor("k_st1", [128, PSL], bf16))
        ps0 = ctx.enter_context(nc.psum_tensor("k_ps0", [128, PSL], f32))
        ps1 = ctx.enter_context(nc.psum_tensor("k_ps1", [128, PSL], f32))

        s_cnt = ctx.enter_context(nc.semaphore("s_cnt"))
        s_ht = [ctx.enter_context(nc.semaphore(f"s_ht{i}")) for i in range(2)]
        s_w = [ctx.enter_context(nc.semaphore(f"s_w{i}")) for i in range(4)]
        s_ms = ctx.enter_context(nc.semaphore("s_ms"))
        s_sc = [ctx.enter_context(nc.semaphore(f"s_sc{i}")) for i in range(4)]
        s_mm = ctx.enter_context(nc.semaphore("s_mm"))
        s_f = [ctx.enter_context(nc.semaphore(f"s_f{i}")) for i in range(4)]
        s_cp = ctx.enter_context(nc.semaphore("s_cp"))
        s_cp2 = ctx.enter_context(nc.semaphore("s_cp2"))
        s_out = ctx.enter_context(nc.semaphore("s_out"))

        nhtch = len(HTB) - 1
        ht_thr = [16 * (c // 2 + 1) for c in range(nhtch)]
        NCH = 2

        with nc.Block() as block:

            @block.sync
            def _(sync):
                sync.dma_start(cnt_sb[:], cnts_d[:]).then_inc(s_cnt, 16)
                nxt = 0
                for c in range(NCALL):
                    # interleave ht chunk loads with the row/idx stream
                    while nxt < nhtch and HTB[nxt] <= 2 * c:
                        if nxt >= 2:
                            sync.wait_ge(s_ht[nxt % 2], 16 * (nxt // 2))
                        lo, hi = 2 * HTB[nxt], 2 * HTB[nxt + 1]
                        sync.dma_start(hta_sb[:, lo:hi], hta_d[:, lo:hi]) \
                            .then_inc(s_ht[nxt % 2], 16)
                        nxt += 1
                    if c >= 4:
                        # wb/ib buffer reuse: scatter c-4 must have drained
                        sync.wait_ge(s_sc[c % 4], 16 * (c // 4))
                    sync.dma_start(wb[:, c % 4], wrows_d[c]).then_inc(s_w[c % 4], 16)
                    sync.dma_start(ib[:, c % 4], idxs_d[c]).then_inc(s_w[c % 4], 16)
                while nxt < nhtch:
                    sync.wait_ge(s_ht[nxt % 2], 16 * (nxt // 2))
                    lo, hi = 2 * HTB[nxt], 2 * HTB[nxt + 1]
                    sync.dma_start(hta_sb[:, lo:hi], hta_d[:, lo:hi]) \
                        .then_inc(s_ht[nxt % 2], 16)
                    nxt += 1
                for c in range(NCH):
                    sl = slice(c * (PSL // NCH), (c + 1) * (PSL // NCH))
                    sync.wait_ge(s_cp, c + 1)
                    sync.dma_start(out_d[0][:, sl], st0[:, sl]).then_inc(s_out, 16)
                for c in range(NCH):
                    sl = slice(c * (PSL // NCH), (c + 1) * (PSL // NCH))
                    sync.wait_ge(s_cp2, c + 1)
                    sync.dma_start(out_d[1][:, sl], st1[:, sl]).then_inc(s_out, 16)
                sync.wait_ge(s_out, 16 * 2 * NCH)

            @block.vector
            def _(vector):
                for c in range(NCALL):
                    rs = (4 * c) % NRING
                    nslot = 4 if c < NCALL - 1 else 2
                    if c >= 4:
                        # ring reuse: PE done with pairs previously in slots
                        vector.wait_ge(s_mm, 2 * c - 6)
                    vector.memset(s16[:, rs:rs + nslot, :, 0], 0.0).then_inc(s_ms, 1)
                for c in range(NCH):
                    sl = slice(c * (PSL // NCH), (c + 1) * (PSL // NCH))
                    vector.wait_ge(s_f[c], 1)
                    vector.tensor_copy(st0[:, sl], ps0[:, sl]).then_inc(s_cp, 1)

            @block.scalar
            def _(scalar):
                for c in range(NCH):
                    sl = slice(c * (PSL // NCH), (c + 1) * (PSL // NCH))
                    scalar.wait_ge(s_f[2 + c], 1)
                    scalar.copy(st1[:, sl], ps1[:, sl]).then_inc(s_cp2, 1)

            @block.gpsimd
            def _(gpsimd):
                from concourse import library_config
                gpsimd.load_library(library_config.mlp)
                r_n = gpsimd.alloc_register("r_cnt")
                gpsimd.wait_ge(s_cnt, 16)
                for c in range(NCALL):
                    rs = (4 * c) % NRING
                    gpsimd.wait_ge(s_w[c % 4], 32 * (c // 4 + 1))
                    gpsimd.wait_ge(s_ms, c + 1)
                    gpsimd.reg_load(r_n, cnt_sb[:1, c:c + 1])
                    oap = s16[:, rs:rs + 2]
                    oap2 = s16[:, rs + 2:rs + 4] if c < NCALL - 1 else dummy[:]
                    gpsimd.dma_scatter_add(
                        oap,
                        wb[:, c % 4],
                        ib[:, c % 4],
                        num_idxs=CAP,
                        num_idxs_reg=r_n,
                        elem_size=ELEM,
                        sbuf_tokens_per_rank=128,
                        parity_reg=0,
                        out_ap_other=oap2,
                    ).then_inc(s_sc[c % 4], 16)

            @block.tensor
            def _(tensor):
                nxt = 0
                for a in range(NPAIR):
                    if nxt < nhtch and a == HTB[nxt]:
                        tensor.wait_ge(s_ht[nxt % 2], ht_thr[nxt])
                        nxt += 1
                    if a % 2 == 0:
                        c = a // 2
                        tensor.wait_ge(s_sc[c % 4], 16 * (c // 4 + 1))
                    rs = (2 * a) % NRING
                    for typ in range(3):
                        v = typ            # ht variant: 0=hhi, 1=hlo, 2=hhi/16
                        d = 0 if typ < 2 else 1
                        for h in range(2):
                            lhsT = hta_sb[:, 2 * a:2 * a + 2, v,
                                          h * 128:(h + 1) * 128]
                            ps = ps0 if h == 0 else ps1
                            for n in range(4):
                                mm = tensor.matmul(
                                    ps[:, n * 512:(n + 1) * 512],
                                    lhsT,
                                    s_sb[:, rs:rs + 2,
                                         n * 512:(n + 1) * 512, d:d + 1],
                                    start=(a == 0 and typ == 0),
                                    stop=(a == NPAIR - 1 and typ == 2),
                                    perf_mode=mybir.MatmulPerfMode.DoubleRow,
                                )
                                if a == NPAIR - 1 and typ == 2 and n % 2 == 1:
                                    mm.then_inc(s_f[h * 2 + n // 2], 1)
                    if a < NPAIR - 1:
                        mm.then_inc(s_mm, 1)

    nc.compile()
    _cache["nc"] = nc
    return nc


def _preprocess(input_ht, ht_index, im_index, weight):
    """Pack 3-variant e4m3 ht and per-core digit-pair scatter rows."""
    q = ht_index.astype(np.int64)
    p = im_index.astype(np.int64)
    w = weight.astype(np.float32)

    # ht stationaries: hta[b, t, v, ch] with v in (hhi, hlo, hhi/16)
    htq = np.asarray(input_ht, np.float32).reshape(CH, Q)
    htT = np.zeros((QP, CH), np.float32)
    htT[:Q] = htq.T
    hhi = htT.astype(E4M3)
    hlo = (htT - hhi.astype(np.float32)).astype(E4M3)
    h16 = (hhi.astype(np.float32) / 16.0).astype(E4M3)
    hta = np.stack([a.reshape(NSTRIPE, 128, CH) for a in (hhi, hlo, h16)],
                   axis=2)                       # [86, 128, 3, 256]
    hta_dev = np.ascontiguousarray(hta.transpose(1, 0, 2, 3))  # [128, 86, 3, 256]

    # accumulate duplicate (q, p) cells, then 8-bit digit-code the sums
    ckey = q * P + p
    uniq, inv = np.unique(ckey, return_inverse=True)
    xsum = np.bincount(inv, weights=w.astype(np.float64)).astype(np.float32)
    q_u = uniq // P
    p_u = uniq % P
    X = np.round(xsum * 128.0).astype(np.int64)
    hi_b = ((X >> 4).astype(np.float32) / 8.0).astype(E4M3).view(np.uint8)
    lo_b = ((X & 15).astype(np.float32) / 8.0).astype(E4M3).view(np.uint8)
    cell = hi_b.astype(np.int16) | (lo_b.astype(np.int16) << 8)

    core = p_u >> 11
    ploc = p_u & (PSL - 1)
    s = q_u >> 7                   # stripe
    b = q_u & 127                  # partition row
    call = s >> 2
    ls = s & 3
    slot_g = (ls & 1) * 64 + (ploc >> 5)
    parity = ls >> 1
    idx16 = ((slot_g * 2 + parity) << 7) | b

    callid = core * NCALL + call
    rowkey = (callid << 15) | idx16
    ruk, rinv = np.unique(rowkey, return_inverse=True)
    R = ruk.shape[0]
    rows = np.zeros((R, ELEM), np.int16)
    rows[rinv, ploc & (ELEM - 1)] = cell

    u_call = (ruk >> 15).astype(np.int64)
    u_idx16 = (ruk & 32767).astype(np.int16)
    counts = np.bincount(u_call, minlength=NCORES * NCALL)
    if counts.max() > CAP:
        raise RuntimeError(f"scatter capacity exceeded: {counts.max()} > {CAP}")
    starts = np.zeros(NCORES * NCALL, np.int64)
    starts[1:] = np.cumsum(counts)[:-1]
    pos = np.arange(R) - starts[u_call]

    wrows = np.zeros((NCORES, NCALL, 128, CAP // 128, ELEM), np.int16)
    u_core = u_call // NCALL
    u_c = u_call % NCALL
    wrows[u_core, u_c, pos % 128, pos // 128, :] = rows

    idxs_flat = np.full((NCORES, NCALL, CAP), -1, np.int16)
    idxs_flat[u_core, u_c, pos] = u_idx16
    idxs_wrapped = idxs_flat.reshape(NCORES, NCALL, CAP // 16, 16) \
                            .transpose(0, 1, 3, 2)
    idxs_dev = np.ascontiguousarray(np.tile(idxs_wrapped, (1, 1, 8, 1)))

    cnts = np.zeros((NCORES, 1, 32), np.int32)
    cnts[:, 0, :NCALL] = counts.reshape(NCORES, NCALL)
    return hta_dev, wrows, idxs_dev, cnts


def kernel(input_ht, ht_index, im_index, weight):
    input_ht = np.asarray(input_ht, dtype=np.float32)
    ht_index = np.asarray(ht_index)
    im_index = np.asarray(im_index)
    weight = np.asarray(weight, dtype=np.float32)
    hta_dev, wrows, idxs_dev, cnts = _preprocess(input_ht, ht_index, im_index, weight)
    nc = _build_nc()
    in_maps = [
        {"hta": hta_dev,
         "wrows": np.ascontiguousarray(wrows[k]),
         "idxs": idxs_dev[k],
         "cnts": cnts[k]}
        for k in range(NCORES)
    ]
    res = bass_utils.run_bass_kernel_spmd(nc, in_maps, core_ids=list(range(NCORES)))
    out = np.empty((CH, P), np.float32)
    for k in range(NCORES):
        out[:, k * PSL:(k + 1) * PSL] = \
            res.results[k]["out"].reshape(CH, PSL).astype(np.float32)
    return out.reshape(B, C, IM_H, IM_W)


# revision 30
# speedup vs baseline: 1.2760x; 1.2760x over previous
"""HT2IM scatter kernel for Trainium2 (8 NeuronCores, SPMD).

Math: out[ch, p] += ht[ch, q] * w  for each vote (q=ht_index[v], p=im_index[v]),
      ch over B*C=256 channels, q < 10980 (HT pixels), p < 16384 (IM pixels).

Device formulation: out[ch, p] = sum_q ht_T[q, ch] * S[q, p] with the sparse
vote-aggregate matrix S[q, p] = sum_v w_v [q_v=q][p_v=p].

Sharding: output pixels split 8 ways (2048 columns per core); every core keeps
the full ht_T (bf16 stationary, SBUF) and a dense fp8-e3m4 copy of its S slice.

S is built DENSE on the host (pure index binning + dtype packing, no float
math beyond summing duplicate-cell weights, same as the reference's
segment-sum semantics) as 86 q-stripes of [128, 2048] e3m4, streamed
HBM->SBUF through a 16-deep buffer ring at full DMA rate (2KB contiguous per
partition per tile).  The PE consumes each stripe with 8 matmuls
(psum[128ch, 512p] += htT[128q, 128ch].T @ S[128q, 512p]) accumulating over
all 86 stripes; moving operand is e3m4 (1 cycle/row), stationary is bf16
(mixed-dtype matmul, verified exact on HW).  S carries 2*w and ht carries
ht/2 (exact exponent shifts) to center w's e3m4 exponent window; accumulation
is fp32 in PSUM.  rel err ~1.3e-2 vs fp32 reference (e3m4 quantization of w).

The kernel is DMA-light (S 21.5MB + ht 5.5MB + out 1MB per core ~= 82us of
DMA) and PE-bound (86*8 matmuls of 512 cols ~= 147us), so the S stream and
the interleaved ht chunks hide completely behind the matmul pipeline.
"""

import numpy as np
import ml_dtypes

import concourse.bass as bass
from concourse import bacc
from concourse import mybir
from concourse import bass_utils

BF16 = ml_dtypes.bfloat16
E3M4 = ml_dtypes.float8_e3m4

B, C = 4, 64
CH = B * C                  # 256 channels
HT_H, HT_W = 183, 60
Q = HT_H * HT_W             # 10980
QP = 11008                  # padded to 86*128
NSTRIPE = 86
IM_H, IM_W = 128, 128
P = IM_H * IM_W             # 16384
NCORES = 8
PSL = P // NCORES           # 2048 pixel columns per core
NRING = 16                  # S-tile SBUF ring depth
GRP = 4                     # stripes per semaphore group (PE waits once per GRP)
NGRP = NSTRIPE // GRP       # 21 full groups... 86 = 21*4+2
# ht chunk boundaries (stripes): small first chunk for fast pipeline fill
HT_BOUNDS = [0, 1, 4] + list(range(12, NSTRIPE, 8)) + [NSTRIPE]

_cache = {}


def _build_nc():
    if "nc" in _cache:
        return _cache["nc"]
    f32 = mybir.dt.float32
    bf16 = mybir.dt.bfloat16
    f8e3 = mybir.dt.float8e3

    nc = bacc.Bacc(None, target_bir_lowering=False)
    ht_d = nc.dram_tensor("ht", [128, NSTRIPE * CH], bf16, kind="ExternalInput")
    s_d = nc.dram_tensor("s", [NSTRIPE, 128, PSL], f8e3, kind="ExternalInput")
    out_d = nc.dram_tensor("out", [2, 128, PSL], bf16, kind="ExternalOutput")

    from contextlib import ExitStack
    ctx = ExitStack()
    with ctx:
        ht_sb = ctx.enter_context(nc.sbuf_tensor("k_htsb", [128, NSTRIPE * CH], bf16))
        s_sb = ctx.enter_context(nc.sbuf_tensor("k_ssb", [128, NRING, PSL], f8e3))
        st0 = ctx.enter_context(nc.sbuf_tensor("k_st0", [128, PSL], bf16))
        st1 = ctx.enter_context(nc.sbuf_tensor("k_st1", [128, PSL], bf16))
        ps0 = ctx.enter_context(nc.psum_tensor("k_ps0", [128, PSL], f32))
        ps1 = ctx.enter_context(nc.psum_tensor("k_ps1", [128, PSL], f32))

        s_ht = [ctx.enter_context(nc.semaphore(f"s_ht{i}")) for i in range(2)]
        s_s = [ctx.enter_context(nc.semaphore(f"s_s{i}")) for i in range(NRING)]
        s_mm = ctx.enter_context(nc.semaphore("s_mm"))
        s_f = [ctx.enter_context(nc.semaphore(f"s_f{i}")) for i in range(4)]
        s_cp = ctx.enter_context(nc.semaphore("s_cp"))
        s_cp2 = ctx.enter_context(nc.semaphore("s_cp2"))
        s_out = ctx.enter_context(nc.semaphore("s_out"))

        # ht chunk of stripe t: largest c with HT_BOUNDS[c] <= t
        nhtch = len(HT_BOUNDS) - 1
        ht_thr = [16 * (c // 2 + 1) for c in range(nhtch)]
        NCH = 2                 # tail copy/store chunks per psum half

        with nc.Block() as block:

            @block.sync
            def _(sync):
                # interleave ht chunks with the S-tile stream so the first
                # matmul starts after ~1 tile and ht never blocks the ring
                nxt_ht = 0
                for t in range(NSTRIPE):
                    if nxt_ht < nhtch and t == HT_BOUNDS[nxt_ht]:
                        if nxt_ht >= 2:
                            # order same-sem ht DMAs (completion is unordered)
                            sync.wait_ge(s_ht[nxt_ht % 2], 16 * (nxt_ht // 2))
                        lo = HT_BOUNDS[nxt_ht] * CH
                        hi = HT_BOUNDS[nxt_ht + 1] * CH
                        sync.dma_start(ht_sb[:, lo:hi], ht_d[:, lo:hi]) \
                            .then_inc(s_ht[nxt_ht % 2], 16)
                        nxt_ht += 1
                    if t >= NRING:
                        # ring reuse: matmuls of stripe t-NRING must be done
                        sync.wait_ge(s_mm, t - NRING + 1)
                    sync.dma_start(s_sb[:, t % NRING], s_d[t]) \
                        .then_inc(s_s[t % NRING], 16)
                for c in range(NCH):
                    sl = slice(c * (PSL // NCH), (c + 1) * (PSL // NCH))
                    sync.wait_ge(s_cp, c + 1)
                    sync.dma_start(out_d[0][:, sl], st0[:, sl]).then_inc(s_out, 16)
                for c in range(NCH):
                    sl = slice(c * (PSL // NCH), (c + 1) * (PSL // NCH))
                    sync.wait_ge(s_cp2, c + 1)
                    sync.dma_start(out_d[1][:, sl], st1[:, sl]).then_inc(s_out, 16)
                sync.wait_ge(s_out, 16 * 2 * NCH)

            @block.tensor
            def _(tensor):
                nxt_ht = 0
                for t in range(NSTRIPE):
                    if nxt_ht < nhtch and t == HT_BOUNDS[nxt_ht]:
                        tensor.wait_ge(s_ht[nxt_ht % 2], ht_thr[nxt_ht])
                        nxt_ht += 1
                    tensor.wait_ge(s_s[t % NRING], 16 * (t // NRING + 1))
                    for h in range(2):
                        lhsT = ht_sb[:, t * CH + h * 128:t * CH + h * 128 + 128]
                        ps = ps0 if h == 0 else ps1
                        for n in range(4):
                            mm = tensor.matmul(
                                ps[:, n * 512:(n + 1) * 512],
                                lhsT,
                                s_sb[:, t % NRING, n * 512:(n + 1) * 512],
                                start=(t == 0),
                                stop=(t == NSTRIPE - 1),
                            )
                            if t == NSTRIPE - 1 and n % 2 == 1:
                                mm.then_inc(s_f[h * 2 + n // 2], 1)
                    if t < NSTRIPE - 1:
                        mm.then_inc(s_mm, 1)

            @block.vector
            def _(vector):
                for c in range(NCH):
                    sl = slice(c * (PSL // NCH), (c + 1) * (PSL // NCH))
                    vector.wait_ge(s_f[c], 1)
                    vector.tensor_copy(st0[:, sl], ps0[:, sl]).then_inc(s_cp, 1)

            @block.scalar
            def _(scalar):
                for c in range(NCH):
                    sl = slice(c * (PSL // NCH), (c + 1) * (PSL // NCH))
                    scalar.wait_ge(s_f[2 + c], 1)
                    scalar.copy(st1[:, sl], ps1[:, sl]).then_inc(s_cp2, 1)

    nc.compile()
    _cache["nc"] = nc
    return nc


def _preprocess(input_ht, ht_index, im_index, weight):
    """Pack ht (bf16 stripe layout, x0.5) and dense per-core S tiles (e3m4, x2)."""
    q = ht_index.astype(np.int64)
    p = im_index.astype(np.int64)
    w = weight.astype(np.float32)

    # ht_T in stripe layout: ht_sb[b, t*256+ch] = 0.5*ht[ch, 128t+b]
    htq = np.asarray(input_ht, np.float32).reshape(CH, Q) * 0.5
    htT = np.zeros((QP, CH), np.float32)
    htT[:Q] = htq.T
    ht_dev = np.ascontiguousarray(
        htT.reshape(NSTRIPE, 128, CH).transpose(1, 0, 2)
           .reshape(128, NSTRIPE * CH)).astype(BF16)

    core = p >> 11
    idx_in_core = q * PSL + (p & (PSL - 1))
    s_dev = np.empty((NCORES, NSTRIPE, 128, PSL), E3M4)
    for k in range(NCORES):
        m = core == k
        dense = np.bincount(idx_in_core[m], weights=w[m],
                            minlength=QP * PSL).astype(np.float32)
        dense *= 2.0
        s_dev[k] = dense.astype(E3M4).reshape(NSTRIPE, 128, PSL)
    return ht_dev, s_dev


def kernel(input_ht, ht_index, im_index, weight):
    input_ht = np.asarray(input_ht, dtype=np.float32)
    ht_index = np.asarray(ht_index)
    im_index = np.asarray(im_index)
    weight = np.asarray(weight, dtype=np.float32)
    ht_dev, s_dev = _preprocess(input_ht, ht_index, im_index, weight)
    nc = _build_nc()
    in_maps = [{"ht": ht_dev, "s": s_dev[k]} for k in range(NCORES)]
    res = bass_utils.run_bass_kernel_spmd(nc, in_maps, core_ids=list(range(NCORES)))
    out = np.empty((CH, P), np.float32)
    for k in range(NCORES):
        out[:, k * PSL:(k + 1) * PSL] = \
            res.results[k]["out"].reshape(CH, PSL).astype(np.float32)
    return out.reshape(B, C, IM_H, IM_W)
